# revision 7
# baseline (speedup 1.0000x reference)
"""Trainium2 Bass kernel for nn_Advection_45526653338038.

out[v] = -sum_{d,grid} central_grad(gv[v], d), gv = [rho, m_ij=(ru_i/rho)*ru_j].

The periodic central differences sum to zero analytically; the reference value
is pure fp32 rounding determined by the exact op/reduction order of the
XLA-Neuron-compiled reference.  This kernel replicates that arithmetic bit for
bit (verified against the compiled reference NEFFs):
  u = ru * RECIP(rho); m = u * ru (IEEE); s = a-b on rolled fields (IEEE);
  the 1/(2dx)=64 scale commutes (power of two) and is applied at the end;
  per (partition=i, var) fold: d-major over 32 sequential 512-elem TR chunks;
  final 128-partition reduction via the PE fp32 ones-matmul.
Sharding: var-columns across cores (chains are independent per (i, var)),
tiny AllReduce to assemble the [128,16] staged columns, final matmul on all.
"""

import numpy as np

import concourse.bass as bass
import concourse.mybir as mybir
from concourse.bass_utils import run_bass_kernel_spmd

F32 = mybir.dt.float32
P = 128
NJ = 128
NK = 128
JK = NJ * NK
NCHUNK = 32
CSZ = 512          # tensor_reduce chunk (4 j-rows)
ACH = 1024         # streaming chunk (8 j-rows)
NACH = JK // ACH   # 16
NSLOT = 16
PADR = NJ + 2
PADC = NK + 2

# core -> (A, B, R, col0, col1); fields 0=rho 1..3=ru, -1=ones
# slot0 = (A*recip(R))*B, slot1 = (B*recip(R))*A ; col = gv var index
CORE_PLAN = [
    (1, 2, 0, 2, 4),      # m01 -> v2 ; m10 -> v4
    (1, 3, 0, 3, 7),      # m02 -> v3 ; m20 -> v7
    (2, 3, 0, 6, 8),      # m12 -> v6 ; m21 -> v8
    (1, 1, 0, 1, None),   # m00 -> v1
    (2, 2, 0, 5, None),   # m11 -> v5
    (3, 3, 0, 9, None),   # m22 -> v9
    (0, -1, -1, 0, None),  # rho -> v0 (R=ones -> recip=1, B=ones)
    (-1, -1, -1, None, None),
]


def build_kernel():
    nc = bass.Bass()

    r_in = nc.dram_tensor("r_in", [P, JK], F32, kind="ExternalInput")
    a_in = nc.dram_tensor("a_in", [P, JK], F32, kind="ExternalInput")
    b_in = nc.dram_tensor("b_in", [P, JK], F32, kind="ExternalInput")
    sel0_in = nc.dram_tensor("sel0_in", [P, NSLOT], F32, kind="ExternalInput")
    sel1_in = nc.dram_tensor("sel1_in", [P, NSLOT], F32, kind="ExternalInput")
    res_out = nc.dram_tensor("res_out", [1, NSLOT], F32, kind="ExternalOutput")
    acc_out = nc.dram_tensor("acc_out", [P, 2], F32, kind="ExternalOutput")

    cc_in = nc.dram_tensor("cc_in", [P, NSLOT], F32)
    cc_out = nc.dram_tensor("cc_out", [P, NSLOT], F32, addr_space="Shared")

    import contextlib
    with contextlib.ExitStack() as _es:
        e = _es.enter_context
        t_r = e(nc.sbuf_tensor([P, 2, ACH], F32))
        t_a = e(nc.sbuf_tensor([P, 2, ACH], F32))
        t_b = e(nc.sbuf_tensor([P, 2, ACH], F32))
        t_rec = e(nc.sbuf_tensor([P, 2, ACH], F32))
        t_u = e(nc.sbuf_tensor([P, 2, ACH], F32))
        pad0 = e(nc.sbuf_tensor([P, PADR, PADC], F32))
        pad1 = e(nc.sbuf_tensor([P, PADR, PADC], F32))
        t_ia = e(nc.sbuf_tensor([P, 2, ACH], F32))
        t_ib = e(nc.sbuf_tensor([P, 2, ACH], F32))
        t_s = e(nc.sbuf_tensor([P, CSZ], F32))
        t_acc = e(nc.sbuf_tensor([P, NSLOT], F32))
        t_stage = e(nc.sbuf_tensor([P, NSLOT], F32))
        t_tmp = e(nc.sbuf_tensor([P, NSLOT], F32))
        t_sel0 = e(nc.sbuf_tensor([P, NSLOT], F32))
        t_sel1 = e(nc.sbuf_tensor([P, NSLOT], F32))
        t_ones = e(nc.sbuf_tensor([P, 1], F32))
        t_res = e(nc.sbuf_tensor([P, NSLOT], F32))
        p_tr = e(nc.psum_tensor([P, 8], F32))
        p_mm = e(nc.psum_tensor([P, NSLOT], F32))
        dsem_a0 = e(nc.semaphore())
        dsem_a1 = e(nc.semaphore())
        dsem_i0 = e(nc.semaphore())
        dsem_i1 = e(nc.semaphore())
        dsem_m = e(nc.semaphore())
        vsem = e(nc.semaphore())
        ssem = e(nc.semaphore())
        msem = e(nc.semaphore())
        csem = e(nc.semaphore())
        block = e(nc.Block())

        nv = 0
        issued_a = {0: 0, 1: 0}
        issued_i = {0: 0, 1: 0}
        issued_m = 0
        dma_plan = []   # {kind, ..., wait_v, sem}
        vec_plan = []   # {kind, ..., wait_sem: (sem_key, count)}
        a_done = {}     # phase-A chunk -> vsem level when its reads are done
        ish_done = {}   # (slot, q) -> vsem level when i-shift tile reads done

        # ---------------- phase A: build padded m fields -----------------
        for q in range(NACH):
            buf = q % 2
            war = a_done.get(q - 2)
            rows = slice(1 + 8 * q, 1 + 8 * (q + 1))
            for t, f in ((t_r, r_in), (t_a, a_in), (t_b, b_in)):
                dma_plan.append(dict(kind="chunk", t=t, buf=buf, f=f, q=q, wait_v=war,
                                     sem=("a", buf)))
                issued_a[buf] += 1
            vec_plan.append(dict(kind="recip", buf=buf,
                                 wait_sem=("a", buf, issued_a[buf] * 16))); nv += 1
            vec_plan.append(dict(kind="mul_u", buf=buf, srcs="ar")); nv += 1
            vec_plan.append(dict(kind="mul_m", buf=buf, pad=0, rows=rows)); nv += 1
            vec_plan.append(dict(kind="mul_u", buf=buf, srcs="br")); nv += 1
            vec_plan.append(dict(kind="mul_m", buf=buf, pad=1, rows=rows)); nv += 1
            a_done[q] = nv

        interior_done = nv
        for pad in (pad0, pad1):
            vec_plan.append(dict(kind="copy", dst=pad[:, 1:129, 0:1], src=pad[:, 1:129, 128:129])); nv += 1
            vec_plan.append(dict(kind="copy", dst=pad[:, 1:129, 129:130], src=pad[:, 1:129, 1:2])); nv += 1
            vec_plan.append(dict(kind="copy", dst=pad[:, 0:1, 0:130], src=pad[:, 128:129, 0:130])); nv += 1
            vec_plan.append(dict(kind="copy", dst=pad[:, 129:130, 0:130], src=pad[:, 1:2, 0:130])); nv += 1
        vec_plan.append(dict(kind="memset", dst=t_acc[:], val=0.0)); nv += 1

        # ---------------- phase C: fold chains ---------------------------
        for slot in (0, 1):
            pad = pad0 if slot == 0 else pad1
            acc_col = t_acc[:, slot:slot + 1]
            # d = 0 (i axis): partition-shifted tiles via SBUF->SBUF DMA
            for q in range(NACH):
                buf = q % 2
                rows = slice(1 + 8 * q, 1 + 8 * (q + 1))
                if (slot, q - 2) in ish_done:
                    war = ish_done[(slot, q - 2)]
                elif slot == 1 and q < 2:
                    war = ish_done[(0, NACH - 2 + q)]
                else:
                    war = interior_done
                # a tile: m[p+1]
                dma_plan.append(dict(kind="ish", t=t_ia, buf=buf, d0=0, dn=127,
                                     pad=pad, s0=1, sn=127, rows=rows, wait_v=war,
                                     sem=("i", buf)))
                dma_plan.append(dict(kind="ish", t=t_ia, buf=buf, d0=127, dn=1,
                                     pad=pad, s0=0, sn=1, rows=rows, wait_v=war,
                                     sem=("i", buf)))
                # b tile: m[p-1]
                dma_plan.append(dict(kind="ish", t=t_ib, buf=buf, d0=1, dn=127,
                                     pad=pad, s0=0, sn=127, rows=rows, wait_v=war,
                                     sem=("i", buf)))
                dma_plan.append(dict(kind="ish", t=t_ib, buf=buf, d0=0, dn=1,
                                     pad=pad, s0=127, sn=1, rows=rows, wait_v=war,
                                     sem=("i", buf)))
                issued_i[buf] += 4
                tile_ready = ("i", buf, issued_i[buf] * 16)
                for cc in range(ACH // CSZ):
                    c = q * (ACH // CSZ) + cc
                    sl = slice(cc * CSZ, (cc + 1) * CSZ)
                    vec_plan.append(dict(kind="stt", a=("it", t_ia, buf, sl),
                                         b=("it", t_ib, buf, sl), c=c,
                                         wait_sem=tile_ready)); nv += 1
                    vec_plan.append(dict(kind="fold", slot=slot, c=c)); nv += 1
                ish_done[(slot, q)] = nv
            # d = 1 (j), d = 2 (k): strided AP reads on the padded field
            for d in (1, 2):
                for c in range(NCHUNK):
                    r0 = 1 + 4 * c
                    if d == 1:
                        a_ap = pad[:, r0 + 1:r0 + 5, 1:129]
                        b_ap = pad[:, r0 - 1:r0 + 3, 1:129]
                    else:
                        a_ap = pad[:, r0:r0 + 4, 2:130]
                        b_ap = pad[:, r0:r0 + 4, 0:128]
                    vec_plan.append(dict(kind="stt", a=("ap", a_ap), b=("ap", b_ap),
                                         c=c, strided=True)); nv += 1
                    vec_plan.append(dict(kind="fold", slot=slot, c=c)); nv += 1

        # ---------------- phase D: stage, allreduce, matmul ---------------
        dma_plan.append(dict(kind="raw", dst=t_sel0[:], src=sel0_in[:], wait_v=None,
                             sem=("m",)))
        dma_plan.append(dict(kind="raw", dst=t_sel1[:], src=sel1_in[:], wait_v=None,
                             sem=("m",)))
        issued_m += 2
        vec_plan.append(dict(kind="ts_mul", dst=t_stage[:], in0=t_sel0[:],
                             scalar=t_acc[:, 0:1],
                             wait_sem=("m", issued_m * 16))); nv += 1
        vec_plan.append(dict(kind="ts_mul", dst=t_tmp[:], in0=t_sel1[:],
                             scalar=t_acc[:, 1:2])); nv += 1
        vec_plan.append(dict(kind="add", dst=t_stage[:], a=t_stage[:], b=t_tmp[:])); nv += 1
        stage_done = nv
        vec_plan.append(dict(kind="memset", dst=t_ones[:], val=1.0)); nv += 1
        ones_done = nv

        sem_map = {("a", 0): dsem_a0, ("a", 1): dsem_a1,
                   ("i", 0): dsem_i0, ("i", 1): dsem_i1, ("m",): dsem_m}

        @block.sync
        def _(sync):
            for op in dma_plan:
                if op.get("wait_v"):
                    sync.wait_ge(vsem, op["wait_v"])
                sem = sem_map[op["sem"]]
                k = op["kind"]
                if k == "chunk":
                    sync.dma_start(out=op["t"][:, op["buf"], :],
                                   in_=op["f"][:, op["q"] * ACH:(op["q"] + 1) * ACH]
                                   ).then_inc(sem, 16)
                elif k == "ish":
                    sync.dma_start(
                        out=op["t"][op["d0"]:op["d0"] + op["dn"], op["buf"], :],
                        in_=op["pad"][op["s0"]:op["s0"] + op["sn"], op["rows"], 1:129],
                    ).then_inc(sem, 16)
                elif k == "raw":
                    sync.dma_start(out=op["dst"], in_=op["src"]).then_inc(sem, 16)
            sync.wait_ge(vsem, stage_done)
            sync.dma_start(out=acc_out[:], in_=t_acc[:, 0:2]).then_inc(dsem_m, 16)
            sync.dma_start(out=cc_in[:], in_=t_stage[:]).then_inc(dsem_m, 16)
            sync.wait_ge(csem, 1)
            sync.dma_start(out=t_tmp[:], in_=cc_out[:]).then_inc(dsem_m, 16)
            sync.wait_ge(ssem, 1)
            sync.dma_start(out=res_out[:], in_=t_res[0:1, :]).then_inc(dsem_m, 16)

        @block.vector
        def _(vector):
            idx = 0
            for op in vec_plan:
                idx += 1
                if idx > 1:
                    vector.wait_ge(vsem, idx - 1)   # full same-engine serialization
                if op.get("wait_sem"):
                    ws = op["wait_sem"]
                    vector.wait_ge(sem_map[ws[:-1]], ws[-1])
                k = op["kind"]
                if k == "recip":
                    ins = vector.reciprocal(t_rec[:, op["buf"], :], t_r[:, op["buf"], :])
                elif k == "mul_u":
                    a = t_a if op["srcs"] == "ar" else t_b
                    ins = vector.tensor_tensor(out=t_u[:, op["buf"], :], in0=a[:, op["buf"], :],
                                               in1=t_rec[:, op["buf"], :], op=mybir.AluOpType.mult)
                elif k == "mul_m":
                    pad = pad0 if op["pad"] == 0 else pad1
                    other = t_b if op["pad"] == 0 else t_a
                    ins = vector.tensor_tensor(out=pad[:, op["rows"], 1:129],
                                               in0=t_u[:, op["buf"], :],
                                               in1=other[:, op["buf"], :],
                                               op=mybir.AluOpType.mult)
                elif k == "copy":
                    ins = vector.tensor_copy(op["dst"], op["src"])
                elif k == "memset":
                    ins = vector.memset(op["dst"], op["val"])
                elif k == "stt":
                    def get(x):
                        if x[0] == "it":
                            _, t, buf, sl = x
                            return t[:, buf, sl]
                        return x[1]
                    c = op["c"] % 8
                    s_out = (t_s[:].rearrange("p (a b) -> p a b", a=4)
                             if op.get("strided") else t_s[:])
                    ins = vector.scalar_tensor_tensor(
                        out=s_out, in0=get(op["a"]), scalar=1.0, in1=get(op["b"]),
                        op0=mybir.AluOpType.bypass, op1=mybir.AluOpType.subtract,
                        accum_out=p_tr[:, c:c + 1])
                elif k == "fold":
                    c = op["c"] % 8
                    acc_col = t_acc[:, op["slot"]:op["slot"] + 1]
                    ins = vector.tensor_tensor(out=acc_col, in0=p_tr[:, c:c + 1],
                                               in1=acc_col, op=mybir.AluOpType.add)
                elif k == "ts_mul":
                    ins = vector.tensor_scalar(out=op["dst"], in0=op["in0"],
                                               scalar1=op["scalar"], scalar2=None,
                                               op0=mybir.AluOpType.mult)
                elif k == "add":
                    ins = vector.tensor_tensor(out=op["dst"], in0=op["a"], in1=op["b"],
                                               op=mybir.AluOpType.add)
                else:
                    raise ValueError(k)
                ins.then_inc(vsem, 1)

        @block.gpsimd
        def _(gpsimd):
            gpsimd.wait_ge(dsem_m, (issued_m + 2) * 16)
            gpsimd.collective_compute(
                "AllReduce",
                mybir.AluOpType.add,
                replica_groups=[list(range(8))],
                ins=[cc_in[:]],
                outs=[cc_out[:]],
            ).then_inc(csem, 1)

        @block.tensor
        def _(tensor):
            tensor.wait_ge(vsem, ones_done)
            tensor.wait_ge(dsem_m, (issued_m + 3) * 16)
            nc.tensor.matmul(out=p_mm[0:1, :], lhsT=t_ones[:, 0:1], rhs=t_tmp[:],
                             start=True, stop=True).then_inc(msem, 1)

        @block.scalar
        def _(scalar):
            scalar.wait_ge(msem, 1)
            scalar.copy(t_res[0:1, :], p_mm[0:1, :]).then_inc(ssem, 1)

    return nc


_NC_CACHE = None


def kernel(x: np.ndarray) -> np.ndarray:
    global _NC_CACHE
    x = np.ascontiguousarray(x, dtype=np.float32)
    rho = x[0].reshape(P, JK)
    ru = [x[1 + i].reshape(P, JK) for i in range(3)]
    ones = np.ones((P, JK), dtype=np.float32)
    fields = {0: rho, 1: ru[0], 2: ru[1], 3: ru[2], -1: ones}

    if _NC_CACHE is None:
        _NC_CACHE = build_kernel()
    nc = _NC_CACHE

    in_maps = []
    for c in range(8):
        A, B, R, col0, col1 = CORE_PLAN[c]
        sel0 = np.zeros((P, NSLOT), np.float32)
        sel1 = np.zeros((P, NSLOT), np.float32)
        if col0 is not None:
            sel0[:, col0] = 1.0
        if col1 is not None:
            sel1[:, col1] = 1.0
        in_maps.append({
            "r_in": fields[R], "a_in": fields[A], "b_in": fields[B],
            "sel0_in": sel0, "sel1_in": sel1,
        })

    res = run_bass_kernel_spmd(nc, in_maps, core_ids=list(range(8)))
    out16 = res.results[0]["res_out"][0]
    return (out16[:10] * np.float32(-64.0)).astype(np.float32)


if __name__ == "__main__":
    import os
    x = np.load("/tmp/x2.npy" if os.path.exists("/tmp/x2.npy") else "/tmp/x.npy")
    print(kernel(x=x))
